# revision 27
# baseline (speedup 1.0000x reference)
"""Multi-head causal self-attention (B=2, T=2048, D=1024, H=16, dk=64) on 8
Trainium2 NeuronCores.

Sharding: batch x head-block. Core c handles batch c//4 and heads
[4*(c%4), 4*(c%4)+4). Each core computes its 4 heads' attention plus the
matching column-block of the output projection; the host sums the 4 partial
outputs per batch and adds the output bias.

v4: all matmul operands bf16 (fp32 PSUM accumulate); PE row-group
concurrency on QK head pairs; software-pipelined emission: projection /
output-projection work is split into ~1-2us units and interleaved between
attention kv-pair groups so the PE FIFO never head-of-line blocks on the
ACT engine (exp), and PSUM-slot waits land right after slot releases.

Per-core device kernel:
  - x^T (host pre-transposed) streams in 512-column chunks, double-prefetched
  - Q^T = (Wq x^T + bq), K^T likewise   -> [dk=256, T] layout (dk on partitions)
  - V   = x Wv^T + bv                   -> [T, dk] natural layout (72-elem
    padded rows), with a ones-column at index dk (softmax denominators free)
  - per head pair, per 512-wide q block:
      S^T[kv,q] = K_h^T.T @ Q_h^T  (PE, kv on partitions, head pairs on
                                    disjoint PE row groups, per-j causal crop)
      P^T = exp(S^T/8)             (ACT; |s|<~8 so no max-subtraction;
                                    two kv tiles per op via [128,2,512] PSUM)
      causal zeroing of kv>q       (GpSimd affine_select, diagonal tiles only)
      out^T[dk+1,q] += [V_h|1].T @ P^T   (PE, PSUM-accumulated over kv tiles)
      y_h^T = out^T[:dk] * bcast(1/denominator)   (fused PSUM->SBUF multiply)
  - out_partial[T, D] = Y^T.T @ Wo_block^T  (interleaved; host adds the 4
    bf16 partials + bo in fp32)
"""

import os
from collections import deque

import numpy as np
import ml_dtypes

import concourse.bacc as bacc
import concourse.mybir as mybir
import concourse.tile as tile
from concourse.bass_utils import run_bass_kernel_spmd

F32 = mybir.dt.float32
BF16 = mybir.dt.bfloat16
F32R = mybir.dt.float32r

B, T, D, H = 2, 2048, 1024, 16
DK = 64                 # head dim
NH_LOC = 4              # heads per core
HD = NH_LOC * DK        # 256 local head dims
N_CORES = 8
QB = 512                # q block width
N_QB = T // QB          # 4
N_KT = T // 128         # 16 kv tiles

MM_MODE = os.environ.get("ATT_MM_MODE", "bf16")
MD = BF16 if MM_MODE == "bf16" else F32R
NPD = ml_dtypes.bfloat16 if MM_MODE == "bf16" else np.float32


def _build():
    nc = bacc.Bacc()

    xT = nc.dram_tensor("xT", [D, T], MD, kind="ExternalInput")
    wqT = nc.dram_tensor("wqT", [D, HD], MD, kind="ExternalInput")
    wkT = nc.dram_tensor("wkT", [D, HD], MD, kind="ExternalInput")
    wvT = nc.dram_tensor("wvT", [D, HD], MD, kind="ExternalInput")
    woT = nc.dram_tensor("woT", [HD, D], MD, kind="ExternalInput")
    # biases host-packed into one [128, 260] f32 tensor: cols 0:2 = bq
    # (dk-major per partition), 2:4 = bk, 4:260 = bv broadcast rows
    bqkv = nc.dram_tensor("bqkv", [128, 4 + HD], F32, kind="ExternalInput")
    out = nc.dram_tensor("out", [T, D], MD, kind="ExternalOutput")

    exp = mybir.ActivationFunctionType.Exp

    with nc.allow_low_precision(reason="bf16 matmul inputs; fp32 accumulate"), \
         tile.TileContext(nc) as tc:
        with tc.tile_pool(name="persist", bufs=1) as persist, \
             tc.tile_pool(name="xc_pool", bufs=3) as xcp, \
             tc.tile_pool(name="att_sb", bufs=4) as asb, \
             tc.tile_pool(name="att_ps", bufs=3, space="PSUM") as aps, \
             tc.tile_pool(name="acc_ps", bufs=2, space="PSUM") as ops:
            # ---- persistent SBUF tensors ----
            wqT_sb = persist.tile([128, 8, HD], MD, tag="wq")
            wkT_sb = persist.tile([128, 8, HD], MD, tag="wk")
            wvT_sb = persist.tile([128, 8, HD], MD, tag="wv")
            woT_sb = persist.tile([128, 2, D], MD, tag="wo")
            bqkv_sb = persist.tile([128, 4 + HD], F32, tag="bqkv")
            bq_sb = bqkv_sb[:, 0:2]
            bk_sb = bqkv_sb[:, 2:4]
            bvb_sb = bqkv_sb[:, 4:4 + HD]
            QT_sb = persist.tile([128, 2, T], MD, tag="QT")
            KT_sb = persist.tile([128, 2, T], MD, tag="KT")
            YT_sb = persist.tile([128, 2, T], MD, tag="YT")
            # V rows padded to 72 elems (144B) so PE stationary reads stay
            # 16B-aligned in bf16; col DK holds the ones column
            V_sb = persist.tile([128, N_KT, NH_LOC, 72], MD, tag="V")

            # ---- parameter loads ----
            xT_re = xT[:].rearrange("(t p) n -> p t n", p=128)
            wq_re = wqT[:].rearrange("(t p) n -> p t n", p=128)
            wk_re = wkT[:].rearrange("(t p) n -> p t n", p=128)
            wv_re = wvT[:].rearrange("(t p) n -> p t n", p=128)
            xcs = {}
            xcs[0] = xcp.tile([128, 8, QB], MD, tag="xc", name="xc")
            xcs[1] = xcp.tile([128, 8, QB], MD, tag="xc", name="xc")
            # DMA descriptor generation (~0.7us per start) is serialized
            # per engine sequencer -- spread the startup loads across five
            # engines so they issue in parallel
            nc.sync.dma_start(out=xcs[0][:, 0:4, :], in_=xT_re[:, 0:4, 0:QB])
            nc.scalar.dma_start(out=wqT_sb[:], in_=wq_re[:])
            nc.gpsimd.dma_start(out=wkT_sb[:], in_=wk_re[:])
            nc.sync.dma_start(out=xcs[0][:, 4:8, :], in_=xT_re[:, 4:8, 0:QB])
            nc.gpsimd.dma_start(out=wvT_sb[:], in_=wv_re[:])
            nc.scalar.dma_start(out=bqkv_sb[:], in_=bqkv[:])
            nc.sync.dma_start(out=xcs[1][:], in_=xT_re[:, :, QB:2 * QB])
            nc.gpsimd.memset(V_sb[:, :, :, DK], 1.0)
            nc.scalar.dma_start(out=woT_sb[:], in_=woT[:].rearrange("(t p) n -> p t n", p=128))

            # ---- work units ----
            def u_projqk(n, src_i, m):
                # one Q-or-K projection chain for chunk n, dk half m
                def run():
                    wT_sb, b_sb, dst = ((wqT_sb, bq_sb, QT_sb),
                                        (wkT_sb, bk_sb, KT_sb))[src_i]
                    ps = aps.tile([128, QB], F32, tag="fps", name="psqk", bufs=2)
                    for k8 in range(8):
                        nc.tensor.matmul(
                            ps[:],
                            wT_sb[:, k8, m * 128:(m + 1) * 128],
                            xcs[n][:, k8, :],
                            start=(k8 == 0), stop=(k8 == 7),
                        )
                    nc.vector.tensor_scalar_add(
                        dst[:, m, n * QB:(n + 1) * QB], ps[:],
                        b_sb[:, m:m + 1],
                    )
                return run

            def u_projv(n, tt):
                def run():
                    t = 4 * n + tt
                    ps = aps.tile([128, QB], F32, tag="fps", name="psv", bufs=2)
                    for k8 in range(8):
                        nc.tensor.matmul(
                            ps[:, 0:HD],
                            xcs[n][:, k8, tt * 128:(tt + 1) * 128],
                            wvT_sb[:, k8, :],
                            start=(k8 == 0), stop=(k8 == 7),
                        )
                    nc.vector.tensor_tensor(
                        V_sb[:, t, :, 0:DK],
                        ps[:, 0:HD].rearrange("p (h d) -> p h d", h=NH_LOC),
                        bvb_sb[:].rearrange("p (h d) -> p h d", h=NH_LOC),
                        op=mybir.AluOpType.add,
                    )
                return run

            def u_xcrelease(n):
                # drop the chunk-n SBUF tile reference once its last
                # projection unit has been emitted
                def run():
                    xcs.pop(n, None)
                return run

            def u_outproj_t(qb, t):
                def run():
                    res = asb.tile([128, D], MD, tag="res", name="res")
                    for n2 in range(2):
                        ps = aps.tile([128, QB], F32, tag="fps", name="pso", bufs=2)
                        for k2 in range(2):
                            nc.tensor.matmul(
                                ps[:],
                                YT_sb[:, k2, t * 128:(t + 1) * 128],
                                woT_sb[:, k2, n2 * QB:(n2 + 1) * QB],
                                start=(k2 == 0), stop=(k2 == 1),
                            )
                        nc.vector.tensor_copy(res[:, n2 * QB:(n2 + 1) * QB], ps[:])
                    nc.sync.dma_start(out=out[t * 128:(t + 1) * 128, :], in_=res[:])
                return run

            def _norm(h, ti, qb, outp):
                base = (h % 2) * 64
                yslice = YT_sb[base:base + 64, ti, qb * QB:(qb + 1) * QB]
                sums_f = asb.tile([1, QB], F32, tag="sums", name="sums")
                nc.vector.tensor_copy(sums_f[:], outp[DK:DK + 1, :])
                recip_f = asb.tile([1, QB], F32, tag="recipf", name="recipf")
                nc.vector.reciprocal_approx_fast(recip_f[:], sums_f[:])
                bc_sb = asb.tile([128, QB], F32, tag="bcs", name="bcs")
                nc.gpsimd.partition_broadcast(bc_sb[:], recip_f[:])
                nc.vector.tensor_tensor(
                    yslice, outp[0:DK, :], bc_sb[base:base + 64, :],
                    op=mybir.AluOpType.mult,
                )

            units = deque()

            def pop_units(k):
                for _ in range(min(k, len(units))):
                    units.popleft()()

            # ---- prologue: minimal proj(0) so attention(0, ti=0) can start ----
            for u in (u_projqk(0, 0, 0), u_projqk(0, 1, 0),
                      u_projv(0, 0), u_projv(0, 1), u_projv(0, 2), u_projv(0, 3)):
                u()
            units.append(u_projqk(0, 0, 1))
            units.append(u_projqk(0, 1, 1))
            units.append(u_xcrelease(0))

            # ---- main loop ----
            # filler assignment balances each attention phase's PE deficit:
            # late phases (many kv pairs, ACT-heavy) get the deferred
            # output projections; V-proj of chunk n may slide into att(n)
            # itself (only its last kv pairs read V(n))
            for qb in range(N_QB):
                if qb + 2 < N_QB:
                    xcs[qb + 2] = xcp.tile([128, 8, QB], MD, tag="xc", name="xc")
                    nc.sync.dma_start(out=xcs[qb + 2][:],
                                      in_=xT_re[:, :, (qb + 2) * QB:(qb + 3) * QB])
                if qb == 0:
                    for m in range(2):
                        units.append(u_projqk(1, 0, m))
                        units.append(u_projqk(1, 1, m))
                    for tt in range(4):
                        units.append(u_projv(1, tt))
                    units.append(u_xcrelease(1))
                elif qb == 1:
                    for m in range(2):
                        units.append(u_projqk(2, 0, m))
                        units.append(u_projqk(2, 1, m))
                    for t in range(0, 4):
                        units.append(u_outproj_t(0, t))
                elif qb == 2:
                    for tt in range(4):
                        units.append(u_projv(2, tt))
                    units.append(u_xcrelease(2))
                    for m in range(2):
                        units.append(u_projqk(3, 0, m))
                        units.append(u_projqk(3, 1, m))
                else:
                    for tt in range(4):
                        units.append(u_projv(3, tt))
                    units.append(u_xcrelease(3))
                    for t in range(4, 12):
                        units.append(u_outproj_t(t // 4, t))

                total_pairs = 2 * 2 * (qb + 1)
                pairs_done = 0
                for ti in range(2):
                    heads = (2 * ti, 2 * ti + 1)
                    outps = {}
                    for h in heads:
                        outps[h] = ops.tile([DK + 1, QB], F32, tag="outp", name="outp")
                    n_kv = 4 * (qb + 1)
                    npairs = n_kv // 2
                    sps = {}

                    def _emit_qk(pj, ti=ti, qb=qb, heads=heads):
                        kt0 = 2 * pj
                        for h in heads:
                            sps[(h, pj)] = aps.tile([128, 2, QB], F32,
                                                    tag="sps", name="sps", bufs=2)
                        q0 = max(kt0 * 128 - qb * QB, 0)
                        for j in range(2):
                            for h in heads:
                                base = (h % 2) * 64
                                nc.tensor.matmul(
                                    sps[(h, pj)][:, j, q0:],
                                    KT_sb[base:base + 64, ti, (kt0 + j) * 128:(kt0 + j + 1) * 128],
                                    QT_sb[base:base + 64, ti, qb * QB + q0:(qb + 1) * QB],
                                    start=True, stop=True,
                                )

                    def _emit_pv(pv, ti=ti, qb=qb, heads=heads, n_kv=n_kv):
                        pj, pTs, q0 = pv
                        kt0 = 2 * pj
                        for h in heads:
                            for j in range(2):
                                nc.tensor.matmul(
                                    outps[h][:, q0:],
                                    V_sb[:, kt0 + j, h, 0:DK + 1],
                                    pTs[h][:, j, q0:],
                                    start=(kt0 + j == 0), stop=(kt0 + j == n_kv - 1),
                                )

                    with tc.high_priority(offset=200):
                        _emit_qk(0)
                    pv_prev = None
                    for pj in range(npairs):
                        kt0 = 2 * pj
                        q0 = max(kt0 * 128 - qb * QB, 0)
                        pTs = {}
                        for h in heads:
                            sp = sps.pop((h, pj))
                            pT = asb.tile([128, 2, QB], MD, tag="pT", bufs=6)
                            pTs[h] = pT
                            nc.scalar.activation(pT[:, :, q0:], sp[:, :, q0:],
                                                 exp, scale=0.125)
                            for j in range(2):
                                r = (kt0 + j) * 128 - qb * QB
                                if r >= 0:
                                    nc.gpsimd.affine_select(
                                        out=pT[:, j, q0:], in_=pT[:, j, q0:],
                                        compare_op=mybir.AluOpType.is_ge,
                                        fill=0.0, base=q0 - r, channel_multiplier=-1,
                                        pattern=[[1, QB - q0]],
                                    )
                        if pj + 1 < npairs:
                            _emit_qk(pj + 1)
                        if pv_prev is not None:
                            _emit_pv(pv_prev)
                        pv_prev = (pj, pTs, q0)
                        pairs_done += 1
                        rem = total_pairs - pairs_done
                        if rem > 0:
                            k = -(-len(units) // rem)  # ceil division
                            pop_units(k)
                        else:
                            pop_units(len(units))
                    _emit_pv(pv_prev)
                    for h in heads:
                        _norm(h, ti, qb, outps[h])

            for t in range(4 * (N_QB - 1), 4 * (N_QB - 1) + 4):
                units.append(u_outproj_t(N_QB - 1, t))
            pop_units(len(units))

    nc.finalize()
    return nc


_NC = None


def _get_nc():
    global _NC
    if _NC is None:
        _NC = _build()
    return _NC


def _shard_inputs(x, wq, bq, wk, bk, wv, bv, wo):
    in_maps = []
    for c in range(N_CORES):
        b = c // 4
        sl = slice((c % 4) * HD, (c % 4 + 1) * HD)
        in_maps.append({
            "xT": np.ascontiguousarray(x[b].T).astype(NPD),
            "wqT": np.ascontiguousarray(wq[sl].T).astype(NPD),
            "wkT": np.ascontiguousarray(wk[sl].T).astype(NPD),
            "wvT": np.ascontiguousarray(wv[sl].T).astype(NPD),
            "woT": np.ascontiguousarray(wo[:, sl].T).astype(NPD),
            "bqkv": np.ascontiguousarray(np.concatenate([
                bq[sl].reshape(2, 128).T,
                bk[sl].reshape(2, 128).T,
                np.broadcast_to(bv[sl][None, :], (128, HD)),
            ], axis=1)).astype(np.float32),
        })
    return in_maps


def kernel(x, wq, bq, wk, bk, wv, bv, wo, bo, _trace=False, **_trace_kw):
    x = np.asarray(x, dtype=np.float32)
    nc = _get_nc()
    in_maps = _shard_inputs(
        x, np.asarray(wq), np.asarray(bq), np.asarray(wk), np.asarray(bk),
        np.asarray(wv), np.asarray(bv), np.asarray(wo))
    res = run_bass_kernel_spmd(nc, in_maps, list(range(N_CORES)),
                               trace=_trace, **_trace_kw)
    parts = [np.asarray(res.results[c]["out"], dtype=np.float32)
             for c in range(N_CORES)]
    bo = np.asarray(bo, dtype=np.float32)
    y = np.stack([
        parts[0] + parts[1] + parts[2] + parts[3] + bo,
        parts[4] + parts[5] + parts[6] + parts[7] + bo,
    ]).astype(np.float32)
    if _trace:
        kernel.last_results = res
    return y
